# revision 15
# baseline (speedup 1.0000x reference)
"""Trainium2 Bass kernel: Conv3d(3->16, k=3, valid) + bias, min over D, softmax over C.

Full inputs: x [16,3,64,64,64], weight [16,3,3,3,3], bias [16].
Output: [16,16,62,62] f32.

Sharding: data-parallel, 2 samples per core across 8 cores.

Per-core algorithm v5 (d-banded bf16 matmul, direct cast loads):
  - Band over D: out partitions m = (dp of 8, co of 16) = 128, contraction
    q = (kw of 3, dr of 10, ci of 3) = 90, kh via 3 PSUM accumulation steps
    (free-dim row offset h'+kh). bf16 matmuls: 1 cycle/row, FWL weight loads.
  - rhs tile per (n, d-block): partition (kw,dr,ci) holds the bf16 (h,w)
    plane x[n,ci,d0+dr,:,:] at flat offset (2-kw) (kw shift baked into
    placement). Loaded as: one gpsimd SWDGE cast-DMA (f32 HBM -> bf16, 16KB
    descriptors) into the kw=1 partitions, then two SBUF->SBUF bf16 copies
    into the kw=0/2 partitions on the sync/scalar HWDGE queues.
  - min over d' merges the 8 d-block PSUMs into bf16 acc[(dp,co), h', w'];
    chunks alternate between a direct DVE min (PSUM f32 operand) and an
    ACT copy->bf16 scratch + DVE 4x-mode bf16 min, balancing both engines.
  - dp-group tree-min: gpsimd-queue DMA copies the upper partition half to
    base 0, DVE bf16 min; 3 steps -> m at partitions 0..15.
  - softmax over co: ACT exp(+bias) -> bf16, all-ones 16x16 matmul for the
    channel sum per 496-col chunk, DVE reciprocal_approx_fast + multiply,
    one fat output DMA per sample.
"""

import sys

for _p in ("/opt/trn_rl_repo",):
    if _p not in sys.path:
        sys.path.insert(0, _p)

import ml_dtypes
import numpy as np

import concourse.bass as bass
import concourse.tile as tile
from concourse import bacc, mybir
from concourse.bass_utils import run_bass_kernel_spmd

# Drop redundant LDWEIGHTS after tile legalization: when consecutive PE
# matmuls reuse the same stationary, the PE array already holds the weights,
# so the repeated Ldweights (which carry no dependency edges -- the matmuls
# keep the graph) can simply be removed from the schedule.
_orig_tile_legalize = tile.tile_legalize


def _tile_legalize_dedup_ldw(obib, nc_):
    out = _orig_tile_legalize(obib, nc_)
    for bb, insts in out.items():
        kept = []
        last_sig = None
        for inst in insts:
            if inst.engine == mybir.EngineType.PE:
                if isinstance(inst, mybir.InstLdweights):
                    sig = str(inst.ins[0]) if inst.ins else None
                    if (
                        sig is not None
                        and sig == last_sig
                        and not inst.descendants
                        and not inst.nosync_dependency_names()
                    ):
                        continue  # redundant reload of the resident weights
                    last_sig = sig
                elif not isinstance(inst, mybir.InstMatmult):
                    last_sig = None
            kept.append(inst)
        if len(kept) != len(insts):
            insts[:] = kept
    return out


tile.tile_legalize = _tile_legalize_dedup_ldw

NS, CIN, CO = 2, 3, 16  # samples per core, in/out channels
D = H = W = 64
DO = HO = WO = 62
SP = HO * WO  # 3844 spatial outputs per (n, co)
DSTARTS = [0, 8, 16, 24, 32, 40, 48, 54]  # d-block starts (last overlaps)
HCH = [(0, 8), (8, 8), (16, 8), (24, 8), (32, 8), (40, 8), (48, 8), (56, 6)]

LAST_EXEC_NS = None

_nc_cache = None


def _build_nc():
    f32 = mybir.dt.float32
    bf16 = mybir.dt.bfloat16
    nc = bacc.Bacc(None, target_bir_lowering=False)
    x = nc.dram_tensor("x", [NS, CIN, D, H, W], f32, kind="ExternalInput")
    lw = nc.dram_tensor("lw", [90, 3, 128], bf16, kind="ExternalInput")
    sel = nc.dram_tensor("sel", [16, 16], bf16, kind="ExternalInput")
    bia = nc.dram_tensor("bia", [16, 1], f32, kind="ExternalInput")
    y = nc.dram_tensor("y", [NS, CO, HO, WO], f32, kind="ExternalOutput")

    CDHW = CIN * D * H * W
    DHW = D * H * W
    HW = H * W

    with tile.TileContext(nc) as tc:
        with (
            tc.tile_pool(name="wpool", bufs=1) as wpool,
            tc.tile_pool(name="bpool", bufs=5) as bpool,
            tc.tile_pool(name="apool", bufs=2) as apool,
            tc.tile_pool(name="work", bufs=2) as work,
            tc.tile_pool(name="cpsum", bufs=6, space="PSUM") as cpsum,
            tc.tile_pool(name="spsum", bufs=2, space="PSUM") as spsum,
        ):
            L = wpool.tile([90, 3, 128], bf16)
            nc.sync.dma_start(out=L[:, :, :], in_=lw[:, :, :])
            SEL = wpool.tile([16, 16], bf16)
            nc.sync.dma_start(out=SEL[:, :], in_=sel[:, :])
            BIA = wpool.tile([16, 1], f32)
            nc.sync.dma_start(out=BIA[:, :], in_=bia[:, :])

            qeng = [nc.sync, nc.scalar]
            for n in range(NS):
                # bf16 acc[(dp,co), h', w'] with 2 pad rows
                acc = apool.tile([128, 64, 64], bf16, tag="acc")
                for bi, d0 in enumerate(DSTARTS):
                    # rhs tile: partition (kw, dr, ci); free = padded flat plane
                    bt = bpool.tile([90, 65, 64], bf16, tag="b")
                    btf = bt.rearrange("p a b -> p (a b)")
                    if bi == 0:
                        # pipeline fill: 3 parallel cast-DMAs from HBM
                        for kw in range(3):
                            nc.gpsimd.dma_start(
                                out=btf[kw * 30 : kw * 30 + 30, (2 - kw) : (2 - kw) + HW],
                                in_=bass.AP(
                                    x,
                                    n * CDHW + d0 * HW,
                                    [[HW, 10], [DHW, CIN], [1, HW]],
                                ),
                            )
                    else:
                        # kw=1 slice straight from HBM with inline f32->bf16
                        # cast, then two shifted SBUF copies for kw=0/2
                        nc.gpsimd.dma_start(
                            out=btf[30:60, 1 : 1 + HW],
                            in_=bass.AP(
                                x, n * CDHW + d0 * HW, [[HW, 10], [DHW, CIN], [1, HW]]
                            ),
                        )
                        qeng[bi % 2].dma_start(
                            out=btf[0:30, 2 : 2 + HW], in_=btf[30:60, 1 : 1 + HW]
                        )
                        qeng[(bi + 1) % 2].dma_start(
                            out=btf[60:90, 0:HW], in_=btf[30:60, 1 : 1 + HW]
                        )
                    for g, chunks in enumerate((HCH[0:3], HCH[3:6], HCH[6:8])):
                        pss = [
                            cpsum.tile([128, 8, 64], f32, tag="cp", name=f"cp{g}_{i_}")
                            for i_ in range(len(chunks))
                        ]
                        for kh in range(3):
                            for (h0, hs), ps in zip(chunks, pss):
                                a0 = (h0 + kh) * 64 + 2
                                nc.tensor.matmul(
                                    ps[:, :hs, :],
                                    L[:, kh, :],
                                    btf[:, a0 : a0 + hs * 64].rearrange(
                                        "p (a b) -> p a b", a=hs
                                    ),
                                    start=(kh == 0),
                                    stop=(kh == 2),
                                )
                        for ci_, ((h0, hs), ps) in enumerate(zip(chunks, pss)):
                          c = 3 * g + ci_
                          if bi == 0:
                            nc.scalar.copy(
                                out=acc[:, h0 : h0 + hs, :], in_=ps[:, :hs, :]
                            )
                          elif (bi + c) % 2 == 0:
                            # direct DVE min against PSUM
                            nc.vector.tensor_tensor(
                                out=acc[:, h0 : h0 + hs, :],
                                in0=acc[:, h0 : h0 + hs, :],
                                in1=ps[:, :hs, :],
                                op=mybir.AluOpType.min,
                            )
                          else:
                            # ACT copies PSUM->bf16, DVE mins in fast 2x/4x mode
                            sc = work.tile([128, 8, 64], bf16, tag="sc", bufs=3)
                            nc.scalar.copy(out=sc[:, :hs, :], in_=ps[:, :hs, :])
                            nc.vector.tensor_tensor(
                                out=acc[:, h0 : h0 + hs, :],
                                in0=acc[:, h0 : h0 + hs, :],
                                in1=sc[:, :hs, :],
                                op=mybir.AluOpType.min,
                            )
                # per-chunk tail: dp-group tree-min (DMA copy down to base 0
                # + DVE min), then exp / channel-sum / reciprocal / multiply /
                # output DMA -- all per h-chunk so the tail pipelines under
                # the remaining conv matmuls.
                for c, (h0, hs) in enumerate(HCH):
                    ts = work.tile([64, 8, 64], bf16, tag="ts", bufs=3)
                    for half in (64, 32, 16):
                        nc.gpsimd.dma_start(
                            out=ts[0:half, :hs, :],
                            in_=acc[half : 2 * half, h0 : h0 + hs, :],
                        )
                        nc.vector.tensor_tensor(
                            out=acc[0:half, h0 : h0 + hs, :],
                            in0=acc[0:half, h0 : h0 + hs, :],
                            in1=ts[0:half, :hs, :],
                            op=mybir.AluOpType.min,
                        )
                    e = work.tile([16, 8, 64], bf16, tag="e", bufs=3)
                    nc.scalar.activation(
                        out=e[:, :hs, :],
                        in_=acc[0:16, h0 : h0 + hs, :],
                        func=mybir.ActivationFunctionType.Exp,
                        bias=BIA[:, 0:1],
                    )
                    ss = spsum.tile([16, 8, 64], f32, tag="ss")
                    nc.tensor.matmul(
                        ss[:, :hs, :],
                        SEL[:, :],
                        e[:, :hs, :],
                        start=True,
                        stop=True,
                    )
                    r = work.tile([16, 8, 64], f32, tag="r", bufs=3)
                    nc.vector.reciprocal_approx_fast(
                        out=r[:, :hs, :], in_=ss[:, :hs, :]
                    )
                    o = work.tile([16, 8, 64], f32, tag="o", bufs=3)
                    nc.vector.tensor_mul(o[:, :hs, :], e[:, :hs, :], r[:, :hs, :])
                    nc.sync.dma_start(
                        out=bass.AP(
                            y,
                            n * CO * SP + h0 * WO,
                            [[SP, 16], [WO, hs], [1, WO]],
                        ),
                        in_=o[:, :hs, 0:WO],
                    )
    nc.finalize()
    return nc


def _host_consts(weight, bias):
    # L[(kw,dr,ci), kh, (dp,co)] = w[co,ci,dr-dp,kh,kw] banded
    lw = np.zeros((90, 3, 128), np.float32)
    for kw in range(3):
        for dr in range(10):
            for ci in range(CIN):
                for kh in range(3):
                    for dp in range(8):
                        kd = dr - dp
                        if 0 <= kd < 3:
                            lw[kw * 30 + dr * 3 + ci, kh, dp * 16 : dp * 16 + 16] = (
                                weight[:, ci, kd, kh, kw]
                            )
    lw = lw.astype(ml_dtypes.bfloat16)
    sel = np.ones((16, 16), ml_dtypes.bfloat16)
    bia = bias.astype(np.float32).reshape(16, 1)
    return lw, sel, bia


def kernel(x, weight, bias, _trace=False):
    global LAST_EXEC_NS, _nc_cache
    x = np.ascontiguousarray(x, dtype=np.float32)
    lw, sel, bia = _host_consts(
        np.asarray(weight, np.float32), np.asarray(bias, np.float32)
    )
    if _nc_cache is None:
        _nc_cache = _build_nc()
    n_cores = 8
    in_maps = [
        {"x": np.ascontiguousarray(x[2 * k : 2 * k + 2]), "lw": lw, "sel": sel, "bia": bia}
        for k in range(n_cores)
    ]
    res = run_bass_kernel_spmd(_nc_cache, in_maps, list(range(n_cores)), trace=_trace)
    LAST_EXEC_NS = res.exec_time_ns
    out = np.concatenate([res.results[k]["y"] for k in range(n_cores)], axis=0)
    return out.astype(np.float32)


if __name__ == "__main__":
    rng = np.random.default_rng(0)
    x = rng.standard_normal((16, 3, 64, 64, 64), dtype=np.float32)
    w = rng.standard_normal((16, 3, 3, 3, 3), dtype=np.float32) / 9.0
    b = (rng.standard_normal(16) * 0.01).astype(np.float32)
    out = kernel(x, w, b)
    print("out", out.shape, out.dtype, out[0, :, 0, 0])
